# revision 25
# baseline (speedup 1.0000x reference)
"""InfoNCE loss kernel for 8 Trainium2 NeuronCores (symmetric-triangle version).

Math (reference): z = concat(z1, z2) [2N, D] row-normalized; sim = z@z.T/TEMP;
self-diagonal masked; loss = mean(-pos + logsumexp(sim, axis=1)).

sim is SYMMETRIC, so only a triangle of the 16x16 grid of 512-wide band
blocks is computed: 136 blocks instead of 256. Core c (with per-core band
rotation slot s -> band (c+s)%16) computes the canonical pattern
  lhs slot 0:  rhs slots 0..8   (slot 0 = self-diagonal block)
  lhs slot 8:  rhs slots 8..15  (slot 8 = self-diagonal block)
which covers every unordered band pair exactly once across the 8 cores.
Each off-diagonal block contributes exp row-sums (fused scalar-engine
accum_out) to its lhs band AND exp column-sums (fp8e5m2 DoubleRow
ones-matmul over the partition axis) to its rhs band. Diagonal blocks are
masked with (1-I) after exp and row-reduced on the vector engine. The host
sums the per-core partial sums, takes ln, and subtracts the exactly-computed
positive dots. This halves the tensor-engine work vs the full-sim version
(302 DoubleRow matmuls/core vs 512).

Tricks kept from the full-sim version: z pre-scaled by 8 before the e4m3
cast (1/64 folded into the exp scale); fp8 DoubleRow 256-deep contraction.
Column-sum matmul emission is deferred past the next pair's first chains so
the PE never stalls waiting on the scalar engine's exp.
"""

from contextlib import ExitStack

import ml_dtypes
import numpy as np

import concourse.bass as bass
import concourse.tile as tile
from concourse import bacc, mybir
from concourse.bass_utils import run_bass_kernel_spmd

N_CORES = 8
N, D = 4096, 1024
ROWS = 2 * N               # 8192 rows of z
NB = 16                    # 512-row bands
BAND = ROWS // NB          # 512
KT = D // 128              # 8 contraction slices (4 DoubleRow pairs)
TEMP = 0.07
INV_T = 1.0 / TEMP
FP8_SCALE = 8.0            # host pre-scale before e4m3 cast
MM_SCALE = INV_T / (FP8_SCALE * FP8_SCALE)

_CACHE = {}


def _build_graph():
    nc = bacc.Bacc("TRN2", target_bir_lowering=False, debug=False, num_devices=N_CORES)
    z = nc.declare_dram_parameter("z", [NB, 128, KT, BAND], mybir.dt.float8e4, isOutput=False)
    rowacc_d = nc.declare_dram_parameter("rowacc", [128, 2, 4, 4], mybir.dt.float32, isOutput=True)
    diagacc_d = nc.declare_dram_parameter("diagacc", [128, 2, 4], mybir.dt.float32, isOutput=True)
    colsum_d = nc.declare_dram_parameter("colsum", [1, 15, BAND], mybir.dt.float32, isOutput=True)

    fp32 = mybir.dt.float32
    bf16 = mybir.dt.bfloat16
    fp8e4 = mybir.dt.float8e4
    fp8e5 = mybir.dt.float8e5
    AF = mybir.ActivationFunctionType
    AX = mybir.AxisListType.X
    DR = mybir.MatmulPerfMode.DoubleRow

    with tile.TileContext(nc) as tc, ExitStack() as ctx:
        zpool = ctx.enter_context(tc.tile_pool(name="z", bufs=1))
        consts = ctx.enter_context(tc.tile_pool(name="consts", bufs=1))
        pspool = ctx.enter_context(tc.tile_pool(name="ps", bufs=3, space="PSUM"))
        cspool = ctx.enter_context(tc.tile_pool(name="cs", bufs=2, space="PSUM"))
        expool = ctx.enter_context(tc.tile_pool(name="ex", bufs=2))
        exdpool = ctx.enter_context(tc.tile_pool(name="exd", bufs=2))
        outpool = ctx.enter_context(tc.tile_pool(name="outp", bufs=1))

        # stage z into SBUF: one [128, KT, 512] fp8 tile per band slot, all on
        # the sync HWDGE queue in slot order so slot 0 (the first block's only
        # dependency) lands first; each tile is split into two dma_starts so
        # the transfer fans out over more DGE queues
        # the HWDGE fans a dma_start over a shape-dependent subset of queues;
        # quartering the earliest-needed bands by partition range engages more
        # queues in parallel so band 0 lands ~3us sooner
        zc = []
        for s in range(NB):
            t = zpool.tile([128, KT, BAND], fp8e4, tag=f"zc{s}", name=f"zc{s}")
            nparts = 4 if s < 3 else (2 if s < 5 else 1)
            w = 128 // nparts
            for q in range(nparts):
                nc.sync.dma_start(
                    out=t[w * q : w * q + w, :, :], in_=z[s, w * q : w * q + w, :, :]
                )
            zc.append(t)

        # warm-up burst: dummy matmuls keep the PE busy through the HAM
        # activity window while the first z tile is in flight, so the real
        # matmul stream starts un-throttled (2.4 GHz, not 1.2)
        warm = consts.tile([128, 64], fp8e4, tag="warm")
        nc.gpsimd.memset(warm[:], 0.0)
        warmps = cspool.tile([128, BAND], fp32, tag="cs", name="warmps")
        for _ in range(16):
            nc.tensor.matmul(warmps[0:64, 0:64], lhsT=warm[:], rhs=warm[:],
                             start=True, stop=True)

        # constants: -1e6 * identity (pre-exp self mask, added into the raw
        # PSUM diagonal so exp flushes it to 0) and a fp8 ones block for the
        # DoubleRow column-sum matmuls
        negeye = consts.tile([128, 128], fp32, tag="negeye")
        nc.gpsimd.memset(negeye[:], -1.0e6)
        nc.gpsimd.affine_select(
            out=negeye[:],
            in_=negeye[:],
            compare_op=mybir.AluOpType.is_equal,
            fill=0.0,
            base=0,
            pattern=[[-1, 128]],
            channel_multiplier=1,
        )
        ones8 = consts.tile([128, 2, 128], fp8e5, tag="ones8")
        nc.gpsimd.memset(ones8[:], 1.0)

        rowacc = outpool.tile([128, 2, 4, 4], fp32, tag="rowacc")
        diagacc = outpool.tile([128, 2, 4], fp32, tag="diagacc")
        cs_sb = outpool.tile([128, 15, BAND], fp32, tag="cs_sb")

        def mm_chain(ps_slice, L, s, mm):
            # [128 rows, 512 cols] block tile: 4 DoubleRow matmuls, K=1024
            for kp in range(4):
                nc.tensor.matmul(
                    ps_slice,
                    lhsT=zc[L][:, 2 * kp : 2 * kp + 2, 128 * mm : 128 * mm + 128],
                    rhs=zc[s][:, 2 * kp : 2 * kp + 2, :],
                    start=(kp == 0),
                    stop=(kp == 3),
                    perf_mode=DR,
                )

        # deferred column-sum emission: tensor-engine instructions execute in
        # program order, so the ones-matmuls (which wait on the scalar engine's
        # exp) are emitted after the NEXT pair's first chains to avoid PE stalls
        pending_cs = []
        cs_state = {"idx": 0, "cur": None}

        def flush_cs():
            for fn in pending_cs:
                fn()
            pending_cs.clear()

        def emit_cs(exq, h):
            # ones-matmul along the partition axis: every output row equals
            # the column sums of the block's 512 rows; row 0 is kept
            ci = cs_state["idx"]
            cs_state["idx"] += 1
            cur = cspool.tile([128, BAND], fp32, tag="cs", name="cs")
            nc.tensor.matmul(
                cur[:],
                lhsT=ones8[:],
                rhs=exq[:, 0:2, h, :],
                start=True,
                stop=False,
                perf_mode=DR,
            )
            nc.tensor.matmul(
                cur[:],
                lhsT=ones8[:],
                rhs=exq[:, 2:4, h, :],
                start=False,
                stop=True,
                perf_mode=DR,
            )
            nc.vector.tensor_copy(cs_sb[0:1, ci, :], cur[0:1, :])

        def do_diag(d, L, fine=False):
            # self block (slot L, slot L): add -1e6 onto the raw PSUM
            # self-diagonal (exp then flushes it to 0), then fused exp +
            # row-sum accumulation on the scalar engine — no post-exp mask
            # or DVE reduce on the tail chain. fine=True runs one matmul
            # chain per 128-row subtile so the last exp waits only on the
            # last quarter of the block
            def tail(ps_slice, mm):
                nc.vector.tensor_add(
                    ps_slice[:, 128 * mm : 128 * mm + 128],
                    ps_slice[:, 128 * mm : 128 * mm + 128],
                    negeye[:],
                )
                exd = exdpool.tile([128, BAND], fp8e5, tag="exd", name="exd")
                nc.scalar.activation(
                    out=exd[:], in_=ps_slice, func=AF.Exp, bias=0.0,
                    scale=MM_SCALE, accum_out=diagacc[:, d, mm : mm + 1],
                )

            if fine:
                for mm in range(4):
                    ps = pspool.tile([128, 2, BAND], fp32, tag="ps", name="ps")
                    mm_chain(ps[:, 0, :], L, L, mm)
                    if mm == 0:
                        flush_cs()
                    tail(ps[:, 0, :], mm)
            else:
                for mp in range(2):
                    ps = pspool.tile([128, 2, BAND], fp32, tag="ps", name="ps")
                    for h in range(2):
                        mm_chain(ps[:, h, :], L, L, 2 * mp + h)
                    if mp == 0:
                        flush_cs()
                    for h in range(2):
                        tail(ps[:, h, :], 2 * mp + h)

        def do_pair(d, L, slot, blocks):
            exq = expool.tile([128, 4, 2, BAND], fp8e5, tag="exq")
            if len(blocks) == 2:
                for mm in range(4):
                    ps = pspool.tile([128, 2, BAND], fp32, tag="ps", name="ps")
                    for h, s_ in enumerate(blocks):
                        mm_chain(ps[:, h, :], L, s_, mm)
                    if mm == 1:
                        flush_cs()
                    # fused exp + row-sum over both blocks' 1024 cols
                    nc.scalar.activation(
                        out=exq[:, mm, :, :], in_=ps[:], func=AF.Exp, bias=0.0,
                        scale=MM_SCALE, accum_out=rowacc[:, d, mm, slot : slot + 1],
                    )
            else:
                for mp in range(2):
                    ps = pspool.tile([128, 2, BAND], fp32, tag="ps", name="ps")
                    for h in range(2):
                        mm_chain(ps[:, h, :], L, blocks[0], 2 * mp + h)
                    if mp == 0:
                        flush_cs()
                    for h in range(2):
                        mm = 2 * mp + h
                        nc.scalar.activation(
                            out=exq[:, mm, 0, :], in_=ps[:, h, :], func=AF.Exp, bias=0.0,
                            scale=MM_SCALE, accum_out=rowacc[:, d, mm, slot : slot + 1],
                        )
            for h in range(len(blocks)):
                pending_cs.append(lambda exq=exq, h=h: emit_cs(exq, h))

        do_diag(0, 0)
        do_pair(0, 0, 0, [1, 2])
        do_pair(0, 0, 1, [3, 4])
        do_pair(0, 0, 2, [5, 6])
        do_pair(0, 0, 3, [7, 8])
        do_pair(1, 8, 0, [9, 10])
        do_pair(1, 8, 1, [11, 12])
        do_pair(1, 8, 2, [13, 14])
        do_pair(1, 8, 3, [15])
        do_diag(1, 8, fine=True)
        flush_cs()

        nc.sync.dma_start(out=rowacc_d[:], in_=rowacc[:])
        nc.sync.dma_start(out=diagacc_d[:], in_=diagacc[:])
        nc.sync.dma_start(out=colsum_d[:], in_=cs_sb[0:1, :, :])

    nc.compile()
    return nc


def _make_in_maps(z1: np.ndarray, z2: np.ndarray):
    z = np.concatenate([z1, z2], axis=0)          # [8192, 1024] f32
    zt = (z.T * FP8_SCALE).astype(np.float32)     # [1024, 8192]
    # [NB, 128, KT, BAND] band-major fp8 tiles: band, k-within-tile, k-tile, col
    zb = np.ascontiguousarray(
        zt.reshape(KT, 128, NB, BAND).transpose(2, 1, 0, 3)
    ).astype(ml_dtypes.float8_e4m3)
    return [
        {"z": np.ascontiguousarray(zb[[(c + s) % NB for s in range(NB)]])}
        for c in range(N_CORES)
    ]


def kernel(z1: np.ndarray, z2: np.ndarray) -> np.ndarray:
    assert z1.shape == (N, D) and z2.shape == (N, D)
    in_maps = _make_in_maps(z1, z2)

    if "nc" not in _CACHE:
        _CACHE["nc"] = _build_graph()
    res = run_bass_kernel_spmd(_CACHE["nc"], in_maps, core_ids=list(range(N_CORES)))

    S = np.zeros(ROWS, np.float64)
    for c in range(N_CORES):
        r = res.results[c]
        ra = np.asarray(r["rowacc"], dtype=np.float64)    # [128, 2, 4, 4]
        da = np.asarray(r["diagacc"], dtype=np.float64)   # [128, 2, 4]
        cs = np.asarray(r["colsum"], dtype=np.float64)[0]  # [15, 512]
        for d, L in ((0, 0), (1, 8)):
            b = (c + L) % NB
            vals = ra[:, d, :, :].sum(axis=2) + da[:, d, :]   # [128 p, 4 m]
            S[BAND * b : BAND * (b + 1)] += vals.T.reshape(BAND)
        for ci, s in enumerate(list(range(1, 9)) + list(range(9, 16))):
            b = (c + s) % NB
            S[BAND * b : BAND * (b + 1)] += cs[ci, :]

    pos = (z1.astype(np.float64) * z2.astype(np.float64)).sum(axis=1) / TEMP
    loss = np.log(S).mean() - pos.mean()
    return np.asarray(loss, dtype=np.float32)


# revision 26
# speedup vs baseline: 1.0147x; 1.0147x over previous
"""InfoNCE loss kernel for 8 Trainium2 NeuronCores (symmetric-triangle version).

Math (reference): z = concat(z1, z2) [2N, D] row-normalized; sim = z@z.T/TEMP;
self-diagonal masked; loss = mean(-pos + logsumexp(sim, axis=1)).

sim is SYMMETRIC, so only a triangle of the 16x16 grid of 512-wide band
blocks is computed: 136 blocks instead of 256. Core c (with per-core band
rotation slot s -> band (c+s)%16) computes the canonical pattern
  lhs slot 0:  rhs slots 0..8   (slot 0 = self-diagonal block)
  lhs slot 8:  rhs slots 8..15  (slot 8 = self-diagonal block)
which covers every unordered band pair exactly once across the 8 cores.
Each off-diagonal block contributes exp row-sums (fused scalar-engine
accum_out) to its lhs band AND exp column-sums (fp8e5m2 DoubleRow
ones-matmul over the partition axis) to its rhs band. Diagonal blocks are
masked with (1-I) after exp and row-reduced on the vector engine. The host
sums the per-core partial sums, takes ln, and subtracts the exactly-computed
positive dots. This halves the tensor-engine work vs the full-sim version
(302 DoubleRow matmuls/core vs 512).

Tricks kept from the full-sim version: z pre-scaled by 8 before the e4m3
cast (1/64 folded into the exp scale); fp8 DoubleRow 256-deep contraction.
Column-sum matmul emission is deferred past the next pair's first chains so
the PE never stalls waiting on the scalar engine's exp.
"""

from contextlib import ExitStack

import ml_dtypes
import numpy as np

import concourse.bass as bass
import concourse.tile as tile
from concourse import bacc, mybir
from concourse.bass_utils import run_bass_kernel_spmd

N_CORES = 8
N, D = 4096, 1024
ROWS = 2 * N               # 8192 rows of z
NB = 16                    # 512-row bands
BAND = ROWS // NB          # 512
KT = D // 128              # 8 contraction slices (4 DoubleRow pairs)
TEMP = 0.07
INV_T = 1.0 / TEMP
FP8_SCALE = 8.0            # host pre-scale before e4m3 cast
MM_SCALE = INV_T / (FP8_SCALE * FP8_SCALE)

_CACHE = {}


def _build_graph():
    nc = bacc.Bacc("TRN2", target_bir_lowering=False, debug=False, num_devices=N_CORES)
    z = nc.declare_dram_parameter("z", [NB, 128, KT, BAND], mybir.dt.float8e4, isOutput=False)
    rowacc_d = nc.declare_dram_parameter("rowacc", [128, 2, 4, 4], mybir.dt.float32, isOutput=True)
    diagacc_d = nc.declare_dram_parameter("diagacc", [128, 2, 4], mybir.dt.float32, isOutput=True)
    colsum_d = nc.declare_dram_parameter("colsum", [1, 15, BAND], mybir.dt.float32, isOutput=True)

    fp32 = mybir.dt.float32
    bf16 = mybir.dt.bfloat16
    fp8e4 = mybir.dt.float8e4
    fp8e5 = mybir.dt.float8e5
    AF = mybir.ActivationFunctionType
    AX = mybir.AxisListType.X
    DR = mybir.MatmulPerfMode.DoubleRow

    with tile.TileContext(nc) as tc, ExitStack() as ctx:
        zpool = ctx.enter_context(tc.tile_pool(name="z", bufs=1))
        consts = ctx.enter_context(tc.tile_pool(name="consts", bufs=1))
        pspool = ctx.enter_context(tc.tile_pool(name="ps", bufs=3, space="PSUM"))
        cspool = ctx.enter_context(tc.tile_pool(name="cs", bufs=2, space="PSUM"))
        expool = ctx.enter_context(tc.tile_pool(name="ex", bufs=2))
        exdpool = ctx.enter_context(tc.tile_pool(name="exd", bufs=2))
        outpool = ctx.enter_context(tc.tile_pool(name="outp", bufs=1))

        # stage z into SBUF: one [128, KT, 512] fp8 tile per band slot, all on
        # the sync HWDGE queue in slot order so slot 0 (the first block's only
        # dependency) lands first; each tile is split into two dma_starts so
        # the transfer fans out over more DGE queues
        # one whole-band dma_start per slot in slot order: every attempted
        # variation (halving, partition-quartering, engine-splitting,
        # dependency-chaining) made the first band arrive later, not earlier
        zc = []
        for s in range(NB):
            t = zpool.tile([128, KT, BAND], fp8e4, tag=f"zc{s}", name=f"zc{s}")
            nc.sync.dma_start(out=t[:], in_=z[s])
            zc.append(t)

        # warm-up burst: dummy matmuls keep the PE busy through the HAM
        # activity window while the first z tile is in flight, so the real
        # matmul stream starts un-throttled (2.4 GHz, not 1.2)
        warm = consts.tile([128, 64], fp8e4, tag="warm")
        nc.gpsimd.memset(warm[:], 0.0)
        warmps = cspool.tile([128, BAND], fp32, tag="cs", name="warmps")
        for _ in range(44):
            nc.tensor.matmul(warmps[0:64, 0:64], lhsT=warm[:], rhs=warm[:],
                             start=True, stop=True)

        # constants: -1e6 * identity (pre-exp self mask, added into the raw
        # PSUM diagonal so exp flushes it to 0) and a fp8 ones block for the
        # DoubleRow column-sum matmuls
        negeye = consts.tile([128, 128], fp32, tag="negeye")
        nc.gpsimd.memset(negeye[:], -1.0e6)
        nc.gpsimd.affine_select(
            out=negeye[:],
            in_=negeye[:],
            compare_op=mybir.AluOpType.is_equal,
            fill=0.0,
            base=0,
            pattern=[[-1, 128]],
            channel_multiplier=1,
        )
        ones8 = consts.tile([128, 2, 128], fp8e5, tag="ones8")
        nc.gpsimd.memset(ones8[:], 1.0)

        rowacc = outpool.tile([128, 2, 4, 4], fp32, tag="rowacc")
        diagacc = outpool.tile([128, 2, 4], fp32, tag="diagacc")
        cs_sb = outpool.tile([128, 15, BAND], fp32, tag="cs_sb")

        def mm_chain(ps_slice, L, s, mm):
            # [128 rows, 512 cols] block tile: 4 DoubleRow matmuls, K=1024
            for kp in range(4):
                nc.tensor.matmul(
                    ps_slice,
                    lhsT=zc[L][:, 2 * kp : 2 * kp + 2, 128 * mm : 128 * mm + 128],
                    rhs=zc[s][:, 2 * kp : 2 * kp + 2, :],
                    start=(kp == 0),
                    stop=(kp == 3),
                    perf_mode=DR,
                )

        # deferred column-sum emission: tensor-engine instructions execute in
        # program order, so the ones-matmuls (which wait on the scalar engine's
        # exp) are emitted after the NEXT pair's first chains to avoid PE stalls
        pending_cs = []
        cs_state = {"idx": 0, "cur": None}

        def flush_cs():
            for fn in pending_cs:
                fn()
            pending_cs.clear()

        def emit_cs(exq, h):
            # ones-matmul along the partition axis: every output row equals
            # the column sums of the block's 512 rows; row 0 is kept
            ci = cs_state["idx"]
            cs_state["idx"] += 1
            cur = cspool.tile([128, BAND], fp32, tag="cs", name="cs")
            nc.tensor.matmul(
                cur[:],
                lhsT=ones8[:],
                rhs=exq[:, 0:2, h, :],
                start=True,
                stop=False,
                perf_mode=DR,
            )
            nc.tensor.matmul(
                cur[:],
                lhsT=ones8[:],
                rhs=exq[:, 2:4, h, :],
                start=False,
                stop=True,
                perf_mode=DR,
            )
            nc.vector.tensor_copy(cs_sb[0:1, ci, :], cur[0:1, :])

        def do_diag(d, L, fine=False):
            # self block (slot L, slot L): add -1e6 onto the raw PSUM
            # self-diagonal (exp then flushes it to 0), then fused exp +
            # row-sum accumulation on the scalar engine — no post-exp mask
            # or DVE reduce on the tail chain. fine=True runs one matmul
            # chain per 128-row subtile so the last exp waits only on the
            # last quarter of the block
            def tail(ps_slice, mm):
                nc.vector.tensor_add(
                    ps_slice[:, 128 * mm : 128 * mm + 128],
                    ps_slice[:, 128 * mm : 128 * mm + 128],
                    negeye[:],
                )
                exd = exdpool.tile([128, BAND], fp8e5, tag="exd", name="exd")
                nc.scalar.activation(
                    out=exd[:], in_=ps_slice, func=AF.Exp, bias=0.0,
                    scale=MM_SCALE, accum_out=diagacc[:, d, mm : mm + 1],
                )

            if fine:
                for mm in range(4):
                    ps = pspool.tile([128, 2, BAND], fp32, tag="ps", name="ps")
                    mm_chain(ps[:, 0, :], L, L, mm)
                    if mm == 0:
                        flush_cs()
                    tail(ps[:, 0, :], mm)
            else:
                for mp in range(2):
                    ps = pspool.tile([128, 2, BAND], fp32, tag="ps", name="ps")
                    for h in range(2):
                        mm_chain(ps[:, h, :], L, L, 2 * mp + h)
                    if mp == 0:
                        flush_cs()
                    for h in range(2):
                        tail(ps[:, h, :], 2 * mp + h)

        def do_pair(d, L, slot, blocks):
            exq = expool.tile([128, 4, 2, BAND], fp8e5, tag="exq")
            if len(blocks) == 2:
                for mm in range(4):
                    ps = pspool.tile([128, 2, BAND], fp32, tag="ps", name="ps")
                    for h, s_ in enumerate(blocks):
                        mm_chain(ps[:, h, :], L, s_, mm)
                    if mm == 1:
                        flush_cs()
                    # fused exp + row-sum over both blocks' 1024 cols
                    nc.scalar.activation(
                        out=exq[:, mm, :, :], in_=ps[:], func=AF.Exp, bias=0.0,
                        scale=MM_SCALE, accum_out=rowacc[:, d, mm, slot : slot + 1],
                    )
            else:
                for mp in range(2):
                    ps = pspool.tile([128, 2, BAND], fp32, tag="ps", name="ps")
                    for h in range(2):
                        mm_chain(ps[:, h, :], L, blocks[0], 2 * mp + h)
                    if mp == 0:
                        flush_cs()
                    for h in range(2):
                        mm = 2 * mp + h
                        nc.scalar.activation(
                            out=exq[:, mm, 0, :], in_=ps[:, h, :], func=AF.Exp, bias=0.0,
                            scale=MM_SCALE, accum_out=rowacc[:, d, mm, slot : slot + 1],
                        )
            for h in range(len(blocks)):
                pending_cs.append(lambda exq=exq, h=h: emit_cs(exq, h))

        do_diag(0, 0)
        do_pair(0, 0, 0, [1, 2])
        do_pair(0, 0, 1, [3, 4])
        do_pair(0, 0, 2, [5, 6])
        do_pair(0, 0, 3, [7, 8])
        do_pair(1, 8, 0, [9, 10])
        do_pair(1, 8, 1, [11, 12])
        do_pair(1, 8, 2, [13, 14])
        do_pair(1, 8, 3, [15])
        do_diag(1, 8, fine=True)
        flush_cs()

        nc.sync.dma_start(out=rowacc_d[:], in_=rowacc[:])
        nc.sync.dma_start(out=diagacc_d[:], in_=diagacc[:])
        nc.sync.dma_start(out=colsum_d[:], in_=cs_sb[0:1, :, :])

    nc.compile()
    return nc


def _make_in_maps(z1: np.ndarray, z2: np.ndarray):
    z = np.concatenate([z1, z2], axis=0)          # [8192, 1024] f32
    zt = (z.T * FP8_SCALE).astype(np.float32)     # [1024, 8192]
    # [NB, 128, KT, BAND] band-major fp8 tiles: band, k-within-tile, k-tile, col
    zb = np.ascontiguousarray(
        zt.reshape(KT, 128, NB, BAND).transpose(2, 1, 0, 3)
    ).astype(ml_dtypes.float8_e4m3)
    return [
        {"z": np.ascontiguousarray(zb[[(c + s) % NB for s in range(NB)]])}
        for c in range(N_CORES)
    ]


def kernel(z1: np.ndarray, z2: np.ndarray) -> np.ndarray:
    assert z1.shape == (N, D) and z2.shape == (N, D)
    in_maps = _make_in_maps(z1, z2)

    if "nc" not in _CACHE:
        _CACHE["nc"] = _build_graph()
    res = run_bass_kernel_spmd(_CACHE["nc"], in_maps, core_ids=list(range(N_CORES)))

    S = np.zeros(ROWS, np.float64)
    for c in range(N_CORES):
        r = res.results[c]
        ra = np.asarray(r["rowacc"], dtype=np.float64)    # [128, 2, 4, 4]
        da = np.asarray(r["diagacc"], dtype=np.float64)   # [128, 2, 4]
        cs = np.asarray(r["colsum"], dtype=np.float64)[0]  # [15, 512]
        for d, L in ((0, 0), (1, 8)):
            b = (c + L) % NB
            vals = ra[:, d, :, :].sum(axis=2) + da[:, d, :]   # [128 p, 4 m]
            S[BAND * b : BAND * (b + 1)] += vals.T.reshape(BAND)
        for ci, s in enumerate(list(range(1, 9)) + list(range(9, 16))):
            b = (c + s) % NB
            S[BAND * b : BAND * (b + 1)] += cs[ci, :]

    pos = (z1.astype(np.float64) * z2.astype(np.float64)).sum(axis=1) / TEMP
    loss = np.log(S).mean() - pos.mean()
    return np.asarray(loss, dtype=np.float32)
